# revision 8
# baseline (speedup 1.0000x reference)
"""Trainium2 Bass kernel for nn_NegF1: distributed -F1 loss over 16.7M elements.

Data-parallel over 8 NeuronCores; each core streams its 2,097,152-element
slice of probs (f32) / lbls (int32) from HBM.

Per [128, F] tile (all comparisons carry fp32-exact signs):
  DVE:  d  = bf16(p - 0.5)          tensor_scalar subtract (2x mode)
  ACT:  lb = bf16(l)                Copy cast, fused accum -> Npos
  DVE:  w  = lb * d                 tensor_mul bf16 (2x mode)
  DVE:  gd = is_gt(d,0), md = max(d,0) [in-place on d],
        gw = is_gt(w,0), mw = max(w,0)   tensor_scalar bf16 (4x mode)
  PE:   ones[128,1].T @ {md, gd, mw, gw, w} accumulated in 5 PSUM banks
        (the TensorEngine is otherwise idle; reductions ride matmuls).

Final per-core sums (f32): Yd=sum(md), Cp=sum(gd), TPd=sum(mw), C=sum(gw),
Sw=sum(w), Npos=sum(l).  Host combine in float64 (exact algebra):
  Sx = Sw + .5*Npos (= sum l*p);  Y = Yd + .5*Cp (= TP+FP);  TP = TPd + .5*C
  FP = Y - TP;  FN = Npos - C - Sx + TP;  f1 with eps=1e-5;  return -f1.
"""

from contextlib import ExitStack

import numpy as np

N_TOTAL = 16777216
N_CORES = 8
M_PER_CORE = N_TOTAL // N_CORES   # 2097152
P = 128                           # SBUF partitions
EPS = 1e-05
MM_N = 512                        # matmul moving free dim (one PSUM bank)

_CACHE = {}


def build_nc(M=M_PER_CORE, F=2048, bufs=3, in_bufs=4, warmup_mms=24,
             dual_queue=True, debug=False):
    import concourse.bacc as bacc
    import concourse.mybir as mybir
    import concourse.tile as tile

    assert M % (P * F) == 0 and F % MM_N == 0
    T = M // (P * F)
    J = F // MM_N

    f32 = mybir.dt.float32
    i32 = mybir.dt.int32
    bf16 = mybir.dt.bfloat16
    Alu = mybir.AluOpType
    Act = mybir.ActivationFunctionType

    nc = bacc.Bacc("TRN2", target_bir_lowering=False, debug=debug,
                   num_devices=N_CORES)

    probs = nc.dram_tensor("probs", [M], f32, kind="ExternalInput")
    lbls = nc.dram_tensor("lbls", [M], i32, kind="ExternalInput")
    # 5 main sums as scalars, plus per-(partition,tile) Npos partials
    out_main = nc.dram_tensor("acc_main", [1, 5], f32, kind="ExternalOutput")
    out_npos = nc.dram_tensor("acc_npos", [P, T], f32, kind="ExternalOutput")

    p3 = probs.ap().rearrange("(t p f) -> t p f", t=T, p=P, f=F)
    l3 = lbls.ap().rearrange("(t p f) -> t p f", t=T, p=P, f=F)

    with tile.TileContext(nc) as tc, ExitStack() as ctx:
        pin = ctx.enter_context(tc.tile_pool(name="pin", bufs=in_bufs))
        lin = ctx.enter_context(tc.tile_pool(name="lin", bufs=in_bufs))
        dpool = ctx.enter_context(tc.tile_pool(name="dpool", bufs=bufs))
        lbpool = ctx.enter_context(tc.tile_pool(name="lbpool", bufs=bufs))
        wpool = ctx.enter_context(tc.tile_pool(name="wpool", bufs=bufs))
        gdpool = ctx.enter_context(tc.tile_pool(name="gdpool", bufs=bufs))
        mwpool = ctx.enter_context(tc.tile_pool(name="mwpool", bufs=bufs))
        gwpool = ctx.enter_context(tc.tile_pool(name="gwpool", bufs=bufs))
        accp = ctx.enter_context(tc.tile_pool(name="accp", bufs=1))
        psump = ctx.enter_context(tc.tile_pool(name="psump", bufs=1,
                                               space="PSUM"))

        acc_npos = accp.tile([P, T], f32)
        accs = accp.tile([1, 5], f32)
        ones = accp.tile([P, 1], bf16)
        nc.vector.memset(ones[:], 1.0)
        neg_half = accp.tile([P, 1], f32)
        nc.vector.memset(neg_half[:], -0.5)

        psums = [psump.tile([1, MM_N], f32, name=f"ps{q}", tag=f"ps{q}")
                 for q in range(5)]

        # Warm the PE HAM clock-gate (1.2 -> 2.4 GHz takes ~3.4us of
        # sustained activity) while the first input DMAs are in flight.
        if warmup_mms:
            wu = accp.tile([P, MM_N], bf16)
            nc.vector.memset(wu[:], 0.0)
            ps_wu = psump.tile([1, MM_N], f32, name="ps_wu", tag="ps_wu")
            for i in range(warmup_mms):
                nc.tensor.matmul(ps_wu[0:1, :], ones[:], wu[:],
                                 start=(i == 0), stop=(i == warmup_mms - 1))

        ldma = nc.scalar if dual_queue else nc.sync
        for t in range(T):
            pt = pin.tile([P, F], f32)
            nc.sync.dma_start(out=pt[:], in_=p3[t])
            lt = lin.tile([P, F], i32)
            ldma.dma_start(out=lt[:], in_=l3[t])

            # d = bf16(p - 0.5); fp32 affine is sign-exact vs (p > 0.5)
            d = dpool.tile([P, F], bf16)
            nc.scalar.activation(d[:], pt[:], Act.Identity, bias=neg_half[:])

            # lb = bf16(l); fused accum -> Npos partials
            lb = lbpool.tile([P, F], bf16)
            nc.scalar.activation(lb[:], lt[:], Act.Copy,
                                 accum_out=acc_npos[:, t:t + 1])

            # w = l * (p - 0.5)  (exact: lb in {0,1})
            w = wpool.tile([P, F], bf16)
            nc.vector.tensor_mul(out=w[:], in0=lb[:], in1=d[:])

            gd = gdpool.tile([P, F], bf16)
            nc.vector.tensor_scalar(out=gd[:], in0=d[:], scalar1=0.0,
                                    scalar2=None, op0=Alu.is_gt)
            # md = max(d, 0) in place (d has no later readers)
            nc.vector.tensor_scalar(out=d[:], in0=d[:], scalar1=0.0,
                                    scalar2=None, op0=Alu.max)
            gw = gwpool.tile([P, F], bf16)
            nc.vector.tensor_scalar(out=gw[:], in0=w[:], scalar1=0.0,
                                    scalar2=None, op0=Alu.is_gt)
            mw = mwpool.tile([P, F], bf16)
            nc.vector.tensor_scalar(out=mw[:], in0=w[:], scalar1=0.0,
                                    scalar2=None, op0=Alu.max)

            # PE reductions: psums[q] += ones.T @ stream_q
            streams = [d, gd, mw, gw, w]  # Yd, Cp, TPd, C, Sw
            for j in range(J):
                sl = slice(j * MM_N, (j + 1) * MM_N)
                for q, src in enumerate(streams):
                    nc.tensor.matmul(psums[q][0:1, :], ones[:], src[:, sl],
                                     start=(t == 0 and j == 0),
                                     stop=(t == T - 1 and j == J - 1))

        # Collapse each [1, MM_N] PSUM accumulator to a scalar in SBUF
        psjunk = accp.tile([1, MM_N], f32)
        for q in range(5):
            nc.vector.tensor_scalar(out=psjunk[:], in0=psums[q][0:1, :],
                                    scalar1=0.0, scalar2=None,
                                    op0=Alu.add, op1=Alu.add,
                                    accum_out=accs[0:1, q:q + 1])

        nc.sync.dma_start(out=out_main.ap(), in_=accs[:])
        nc.sync.dma_start(out=out_npos.ap(), in_=acc_npos[:])

    nc.compile()
    return nc, T


def get_nc():
    if "nc" not in _CACHE:
        _CACHE["nc"] = build_nc()
    return _CACHE["nc"]


def run_device(probs, lbls, trace=False, **run_kwargs):
    """Run the SPMD kernel; returns (per-core result dicts, BassKernelResults)."""
    from concourse import bass_utils

    nc, _ = get_nc()
    probs = np.ascontiguousarray(probs, dtype=np.float32)
    lbls = np.ascontiguousarray(lbls, dtype=np.int32)
    assert probs.shape == (N_TOTAL,) and lbls.shape == (N_TOTAL,)
    M = M_PER_CORE
    in_maps = [
        {"probs": probs[c * M:(c + 1) * M], "lbls": lbls[c * M:(c + 1) * M]}
        for c in range(N_CORES)
    ]
    res = bass_utils.run_bass_kernel_spmd(
        nc, in_maps, core_ids=list(range(N_CORES)), trace=trace, **run_kwargs)
    return res.results, res


def combine(results):
    """Combine per-core partial sums into the final -f1 scalar."""
    Yd = Cp = TPd = C = Sw = Npos = 0.0
    for r in results:
        am = np.asarray(r["acc_main"], dtype=np.float64).reshape(5)
        Yd += am[0]
        Cp += am[1]
        TPd += am[2]
        C += am[3]
        Sw += am[4]
        Npos += np.asarray(r["acc_npos"], dtype=np.float64).sum()

    Sx = Sw + 0.5 * Npos
    Y = Yd + 0.5 * Cp
    TP = TPd + 0.5 * C
    FP = Y - TP
    FN = Npos - C - Sx + TP
    precision = (TP + EPS) / (TP + FP + EPS)
    recall = (TP + EPS) / (TP + FN + EPS)
    f1 = 2.0 * precision * recall / (precision + recall)
    return np.float32(-f1)


def kernel(probs, lbls):
    results, _ = run_device(probs, lbls)
    return np.asarray(combine(results), dtype=np.float32)


if __name__ == "__main__":
    rng = np.random.default_rng(0)
    p = rng.uniform(0, 1, N_TOTAL).astype(np.float32)
    l = rng.integers(0, 2, N_TOTAL).astype(np.int32)
    out = kernel(p, l)
    print("kernel output:", out)


# revision 11
# speedup vs baseline: 1.1936x; 1.1936x over previous
"""Trainium2 Bass kernel for nn_NegF1: distributed -F1 loss over 16.7M elements.

Data-parallel over 8 NeuronCores; each core streams its 2,097,152-element
slice of probs (f32) / lbls (int32) from HBM.

Per [128, F] tile, DVE writes three bf16 planes into one interleaved
"comb" buffer laid out [128][chunk c][slot s][128] (chunk = 128 columns):
  slot 0: d  = bf16(p - 0.5)     tensor_scalar subtract (fp32-exact sign)
  slot 1: md = max(d, 0)
  slot 2: gd = is_gt(d, 0)
ACT casts lb = bf16(l) with fused accum -> Npos.

The idle TensorEngine does every reduction:
  - diag trick: lhsT = lb chunk [128,128], rhs = comb chunk [128, 3*128],
    accumulated into one PSUM tile [128, 384].  diag of block s gives the
    l-masked sums: Sw = sum(l*d), TPd = sum(l*md), C = sum(l*gd).
  - ones trick: lhsT = ones [128,1], rhs = comb slots 1:3, accumulated into
    PSUM [1, 512]: Yd = sum(md), Cp = sum(gd).

Host combine (float64, exact algebra):
  Sx = Sw + .5*Npos (= sum l*p);  Y = Yd + .5*Cp (= TP+FP);  TP = TPd + .5*C
  FP = Y - TP;  FN = Npos - C - Sx + TP;  f1 with eps=1e-5;  return -f1.
"""

from contextlib import ExitStack

import numpy as np

N_TOTAL = 16777216
N_CORES = 8
M_PER_CORE = N_TOTAL // N_CORES   # 2097152
P = 128                           # SBUF partitions
EPS = 1e-05
MM_N = 512                        # ones-matmul moving free dim
CH = 128                          # diag chunk columns

_CACHE = {}


def build_nc(M=M_PER_CORE, F=2048, bufs=3, in_bufs=4, warmup_mms=24,
             dual_queue=True, mode="diag", debug=False):
    import concourse.bacc as bacc
    import concourse.mybir as mybir
    import concourse.tile as tile

    assert M % (P * F) == 0 and F % MM_N == 0 and F % CH == 0
    T = M // (P * F)
    NC = F // CH                  # chunks per tile

    f32 = mybir.dt.float32
    i32 = mybir.dt.int32
    bf16 = mybir.dt.bfloat16
    Alu = mybir.AluOpType
    Act = mybir.ActivationFunctionType

    nc = bacc.Bacc("TRN2", target_bir_lowering=False, debug=debug,
                   num_devices=N_CORES)

    probs = nc.dram_tensor("probs", [M], f32, kind="ExternalInput")
    lbls = nc.dram_tensor("lbls", [M], i32, kind="ExternalInput")
    out_diag = nc.dram_tensor("out_diag", [P, 3 * CH], f32,
                              kind="ExternalOutput")
    out_ones = nc.dram_tensor("out_ones", [1, MM_N], f32,
                              kind="ExternalOutput")
    out_npos = nc.dram_tensor("acc_npos", [P, T], f32, kind="ExternalOutput")

    p3 = probs.ap().rearrange("(t p f) -> t p f", t=T, p=P, f=F)
    l3 = lbls.ap().rearrange("(t p f) -> t p f", t=T, p=P, f=F)

    with tile.TileContext(nc) as tc, ExitStack() as ctx:
        pin = ctx.enter_context(tc.tile_pool(name="pin", bufs=in_bufs))
        lin = ctx.enter_context(tc.tile_pool(name="lin", bufs=in_bufs))
        lbpool = ctx.enter_context(tc.tile_pool(name="lbpool", bufs=bufs))
        cpool = ctx.enter_context(tc.tile_pool(name="cpool", bufs=bufs))
        accp = ctx.enter_context(tc.tile_pool(name="accp", bufs=1))
        psump = ctx.enter_context(tc.tile_pool(name="psump", bufs=1,
                                               space="PSUM"))

        acc_npos = accp.tile([P, T], f32)
        ones = accp.tile([P, 1], bf16)
        nc.vector.memset(ones[:], 1.0)

        ps_diag = psump.tile([P, 3 * CH], f32)
        ps_ones = psump.tile([1, MM_N], f32)

        # Warm the PE HAM clock-gate (1.2 -> 2.4 GHz needs ~3.4us sustained)
        # while the first input DMAs are in flight.
        if warmup_mms:
            wu = accp.tile([P, MM_N], bf16)
            nc.vector.memset(wu[:], 0.0)
            ps_wu = psump.tile([1, MM_N], f32)
            for i in range(warmup_mms):
                nc.tensor.matmul(ps_wu[0:1, :], ones[:], wu[:],
                                 start=(i == 0), stop=(i == warmup_mms - 1))

        ldma = nc.scalar if dual_queue else nc.sync
        for t in range(T):
            pt = pin.tile([P, F], f32)
            nc.sync.dma_start(out=pt[:], in_=p3[t])
            lt = lin.tile([P, F], i32)
            ldma.dma_start(out=lt[:], in_=l3[t])

            # lb = bf16(l); fused accum -> Npos partials
            lb = lbpool.tile([P, F], bf16)
            nc.scalar.activation(lb[:], lt[:], Act.Copy,
                                 accum_out=acc_npos[:, t:t + 1])

            comb = cpool.tile([P, 3 * F], bf16)
            c4 = comb[:].rearrange("p (c s j) -> p c s j", c=NC, s=3, j=CH)
            pt4 = pt[:].rearrange("p (c j) -> p c j", c=NC, j=CH)

            # slot 0: d = bf16(p - 0.5)
            nc.vector.tensor_scalar(out=c4[:, :, 0, :], in0=pt4,
                                    scalar1=0.5, scalar2=None,
                                    op0=Alu.subtract)
            # slot 1: md = max(d, 0)
            nc.vector.tensor_scalar(out=c4[:, :, 1, :], in0=c4[:, :, 0, :],
                                    scalar1=0.0, scalar2=None, op0=Alu.max)
            # slot 2: gd = is_gt(d, 0)
            nc.vector.tensor_scalar(out=c4[:, :, 2, :], in0=c4[:, :, 0, :],
                                    scalar1=0.0, scalar2=None, op0=Alu.is_gt)

            # diag reductions: ps_diag += lb_c.T @ comb_c
            for c in range(NC):
                nc.tensor.matmul(
                    ps_diag[:, :], lb[:, c * CH:(c + 1) * CH],
                    c4[:, c, :, :],
                    start=(t == 0 and c == 0),
                    stop=(t == T - 1 and c == NC - 1))

            # ones reductions over slots 1:3 (md, gd), two chunks per MM
            for c in range(0, NC, 2):
                nc.tensor.matmul(
                    ps_ones[0:1, :], ones[:], c4[:, c:c + 2, 1:3, :],
                    start=(t == 0 and c == 0),
                    stop=(t == T - 1 and c == NC - 2))

        # PSUM -> SBUF -> DRAM
        diag_sb = accp.tile([P, 3 * CH], f32)
        nc.vector.tensor_copy(out=diag_sb[:], in_=ps_diag[:, :])
        ones_sb = accp.tile([1, MM_N], f32)
        nc.vector.tensor_copy(out=ones_sb[:], in_=ps_ones[0:1, :])

        nc.sync.dma_start(out=out_diag.ap(), in_=diag_sb[:])
        nc.sync.dma_start(out=out_ones.ap(), in_=ones_sb[:])
        nc.sync.dma_start(out=out_npos.ap(), in_=acc_npos[:])

    nc.compile()
    return nc, T


def get_nc():
    if "nc" not in _CACHE:
        _CACHE["nc"] = build_nc()
    return _CACHE["nc"]


def run_device(probs, lbls, trace=False, **run_kwargs):
    """Run the SPMD kernel; returns (per-core result dicts, BassKernelResults)."""
    from concourse import bass_utils

    nc, _ = get_nc()
    probs = np.ascontiguousarray(probs, dtype=np.float32)
    lbls = np.ascontiguousarray(lbls, dtype=np.int32)
    assert probs.shape == (N_TOTAL,) and lbls.shape == (N_TOTAL,)
    M = M_PER_CORE
    in_maps = [
        {"probs": probs[c * M:(c + 1) * M], "lbls": lbls[c * M:(c + 1) * M]}
        for c in range(N_CORES)
    ]
    res = bass_utils.run_bass_kernel_spmd(
        nc, in_maps, core_ids=list(range(N_CORES)), trace=trace, **run_kwargs)
    return res.results, res


def combine(results):
    """Combine per-core partial sums into the final -f1 scalar."""
    Yd = Cp = TPd = C = Sw = Npos = 0.0
    for r in results:
        dg = np.asarray(r["out_diag"], dtype=np.float64).reshape(P, 3, CH)
        Sw += np.trace(dg[:, 0, :])
        TPd += np.trace(dg[:, 1, :])
        C += np.trace(dg[:, 2, :])
        on = np.asarray(r["out_ones"], dtype=np.float64).reshape(2, 2, CH)
        Yd += on[:, 0, :].sum()
        Cp += on[:, 1, :].sum()
        Npos += np.asarray(r["acc_npos"], dtype=np.float64).sum()

    Sx = Sw + 0.5 * Npos
    Y = Yd + 0.5 * Cp
    TP = TPd + 0.5 * C
    FP = Y - TP
    FN = Npos - C - Sx + TP
    precision = (TP + EPS) / (TP + FP + EPS)
    recall = (TP + EPS) / (TP + FN + EPS)
    f1 = 2.0 * precision * recall / (precision + recall)
    return np.float32(-f1)


def kernel(probs, lbls):
    results, _ = run_device(probs, lbls)
    return np.asarray(combine(results), dtype=np.float32)


if __name__ == "__main__":
    rng = np.random.default_rng(0)
    p = rng.uniform(0, 1, N_TOTAL).astype(np.float32)
    l = rng.integers(0, 2, N_TOTAL).astype(np.int32)
    out = kernel(p, l)
    print("kernel output:", out)
